# revision 6
# baseline (speedup 1.0000x reference)
"""Grouped batched matmul: out[b,m,o] = sum_i x[b,m,i] * w[m,i,o].

Shapes (full): x [8192, 16, 256] f32, w [16, 256, 256] f32 -> out [8192, 16, 256] f32.
Data-parallel over batch across 8 NeuronCores (1024 rows/core), weights replicated.

Per-core device kernel (SPMD, identical program):
  - weights loaded once to SBUF as 32 tiles [128i, 256o] (m x k-half)
  - loop over 8 batch tiles of 128 rows:
      big 2MiB DMA load x[128, 16, 256]
      per m: PE-transpose the two [128b,128i] halves -> [128i,128b] (fp32),
             DVE copy PSUM->SBUF, two accumulating fp32 matmuls
             (lhsT = xT (stationary), rhs = w[m] half [128i, 256o] moving)
             -> PSUM [128b, 256o], ACT copy -> staging, 2MiB DMA store
"""

import numpy as np
from contextlib import ExitStack

import concourse.bass as bass
import concourse.tile as tile
import concourse.mybir as mybir
from concourse import bacc
from concourse.bass import ts
from concourse.bass_utils import run_bass_kernel_spmd
from concourse.masks import make_identity

BATCH, M, D_IN, D_OUT = 8192, 16, 256, 256
N_CORES = 8
P = 128  # partitions
F32 = mybir.dt.float32


def build_program(b_per_core: int) -> bass.Bass:
    nc = bacc.Bacc("TRN2", target_bir_lowering=False, debug=False)

    x_ap = nc.dram_tensor("x", [b_per_core, M, D_IN], F32, kind="ExternalInput").ap()
    w_ap = nc.dram_tensor("w", [M, D_IN, D_OUT], F32, kind="ExternalInput").ap()
    o_ap = nc.dram_tensor("out", [b_per_core, M, D_OUT], F32, kind="ExternalOutput").ap()

    n_btiles = b_per_core // P
    KT = D_IN // P  # 2 k-tiles

    with tile.TileContext(nc) as tc, ExitStack() as ctx:
        const_pool = ctx.enter_context(tc.tile_pool(name="const", bufs=1))
        w_pool = ctx.enter_context(tc.tile_pool(name="w", bufs=1))
        x_pool = ctx.enter_context(tc.tile_pool(name="x", bufs=3))
        o_pool = ctx.enter_context(tc.tile_pool(name="o", bufs=3))
        xt_pool = ctx.enter_context(tc.tile_pool(name="xt", bufs=4))
        pst_pool = ctx.enter_context(tc.tile_pool(name="pst", bufs=4, space="PSUM"))
        pso_pool = ctx.enter_context(tc.tile_pool(name="pso", bufs=4, space="PSUM"))

        ident = const_pool.tile([P, P], F32)
        make_identity(nc, ident[:])

        # Resident weights: [128i, (m, k) , 256o] — single DMA so downstream
        # matmuls wait on one semaphore, not 32.
        w_sb = w_pool.tile([P, M * KT, D_OUT], F32)
        nc.sync.dma_start(
            out=w_sb[:], in_=w_ap.rearrange("m (k p) o -> p (m k) o", p=P)
        )

        for bt in range(n_btiles):
            xt = x_pool.tile([P, M, D_IN], F32)
            nc.sync.dma_start(out=xt[:], in_=x_ap[ts(bt, P)])
            ot = o_pool.tile([P, M, D_OUT], F32)

            for m in range(M):
                # transpose the two 128x128 halves of x[:, m, :] on PE
                xts = xt_pool.tile([P, KT, P], F32)
                for k in range(KT):
                    ps_t = pst_pool.tile([P, P], F32)
                    nc.tensor.transpose(ps_t[:], xt[:, m, ts(k, P)], ident[:])
                    nc.vector.tensor_copy(out=xts[:, k, :], in_=ps_t[:])

                ps_o = pso_pool.tile([P, D_OUT], F32)
                for k in range(KT):
                    nc.tensor.matmul(
                        ps_o[:],
                        lhsT=xts[:, k, :],
                        rhs=w_sb[:, m * KT + k, :],
                        start=(k == 0),
                        stop=(k == KT - 1),
                    )
                nc.scalar.copy(out=ot[:, m, :], in_=ps_o[:])

            nc.sync.dma_start(out=o_ap[ts(bt, P)], in_=ot[:])

    nc.compile()
    return nc


def _run(x: np.ndarray, weights: np.ndarray, trace: bool = False):
    b_per_core = x.shape[0] // N_CORES
    nc = build_program(b_per_core)
    shards = np.split(x, N_CORES, axis=0)
    in_maps = [
        {"x": np.ascontiguousarray(s), "w": np.ascontiguousarray(weights)}
        for s in shards
    ]
    res = run_bass_kernel_spmd(nc, in_maps, list(range(N_CORES)), trace=trace)
    out = np.concatenate([r["out"] for r in res.results], axis=0)
    return out, res


def kernel(x: np.ndarray, weights: np.ndarray) -> np.ndarray:
    out, _ = _run(np.asarray(x), np.asarray(weights), trace=False)
    return out
